# revision 61
# baseline (speedup 1.0000x reference)
"""Trainium2 Bass kernel for nn_GATsimple (4-layer GAT + graph readout).

Self-contained: takes the FULL inputs from setup_inputs(), shards across 8
NeuronCores internally (data-parallel over dst-node ranges, 16 graphs/core),
runs a Bass/Tile kernel via run_bass_kernel_spmd, and returns the FULL
[128, 1] output.

v2 design (bf16 pipeline):
  - per-layer node table row (bf16): [h (fo) | s (4 x f32, bitcast as 8 bf16
    slots) | pad to 128-multiple]; d-scores never leave the core (dst-local).
  - node matmul is fused per 128-dst window: as soon as a window's xn (post
    ELU) is ready, it is transposed (hi/lo bf16 split for f32-accurate
    scores), multiplied by the next layer's weights, and its table rows are
    stored; AllGathers run on window-chunks [5,5,5,2] overlapping the edge
    phase of the current layer.
  - edge phase per window: dma_gather of bf16 rows, per-tile one-hot S
    matrices (host-precomputed, SBUF-resident bf16) used for both d-score
    expansion (S^T @ dwin) and the segment-sum (S^T @ [msg|ee]).
  - self-loops bypass the gather: diagonal identity matmul with ee_self
    computed from local s/d.
"""

import os
import sys

import ml_dtypes
import numpy as np

for _p in ("/opt/trn_rl_repo", "/root/.axon_site/_ro/trn_rl_repo"):
    if os.path.isdir(_p) and _p not in sys.path:
        sys.path.append(_p)

import concourse.bass as bass
import concourse.bacc as bacc
import concourse.mybir as mybir
import concourse.tile as tile
from concourse.bass_utils import run_bass_kernel_spmd

F32 = mybir.dt.float32
BF16 = mybir.dt.bfloat16
I16 = mybir.dt.int16

# ---------------------------------------------------------------- problem cfg

N_CORES = 8
HEADS = 4
DUMMY_S = -100.0  # dummy-source s-score; exp(lrelu(-100+d)) ~ 0, h = 0


def rnd128(x):
    return ((x + 127) // 128) * 128


class Cfg:
    def __init__(self, n_nodes, npg, in_feat, layer_out, n_cores=N_CORES):
        assert n_nodes % n_cores == 0
        self.n_nodes = n_nodes
        self.npg = npg
        self.n_cores = n_cores
        self.npc = n_nodes // n_cores
        self.nblk = (self.npc + 127) // 128
        self.npc_pad = self.nblk * 128
        self.dummy = n_cores * self.npc_pad  # global dummy row id
        self.nrows = self.dummy + 1
        self.in_feat = in_feat
        self.layer_out = layer_out
        self.f_out = [HEADS * c for c in layer_out]  # [512, 256, 128, 64]
        self.f_in = [in_feat] + self.f_out[:-1]
        self.n_layers = len(layer_out)
        self.gpc = self.npc // npg
        assert self.npc % npg == 0
        # bf16 table row sizes (128-multiples; h | s-f32-bitcast(8) | pad)
        self.rp = [rnd128(f + 8) for f in self.f_out]  # [640, 384, 256, 128]
        # AllGather window chunks (small tail chunks shrink the exposed
        # AG latency at each layer boundary)
        self.cwin = [6, 6, 4, 1]
        assert sum(self.cwin) == self.nblk
        self.cstart = np.concatenate([[0], np.cumsum(self.cwin)]).tolist()
        # global row base of each chunk (8 cores interleaved per chunk)
        self.cbase = [0]
        for cw in self.cwin[:-1]:
            self.cbase.append(self.cbase[-1] + n_cores * cw * 128)

    def glob_row(self, node):
        """Global padded table row id for node (chunk-major AG layout)."""
        c = node // self.npc
        r = node % self.npc
        w = r // 128
        ch = np.searchsorted(np.array(self.cstart), w, side="right") - 1
        return (
            self.cbase[ch]
            + c * self.cwin[ch] * 128
            + (w - self.cstart[ch]) * 128
            + (r - w * 128)
        )


def default_cfg():
    return Cfg(n_nodes=17024, npg=133, in_feat=64, layer_out=[128, 64, 32, 16])


# ------------------------------------------------------------ host preprocess


def preprocess_edges(cfg, edge_index):
    """Bucket real edges (NO appended self-loops; those are handled by the
    in-kernel diagonal path) by (dst core, 128-dst-window). Returns (tg,
    per_core) with per-core gidx (int16 global row ids, chunk-major) and
    one-hot S matrices [128, 128*ttot] bf16."""
    src = edge_index[0].astype(np.int64)
    dst = edge_index[1].astype(np.int64)
    core = dst // cfg.npc
    win = (dst % cfg.npc) // 128
    key = core * cfg.nblk + win
    order = np.argsort(key, kind="stable")
    src, dst, key = src[order], dst[order], key[order]
    nbuck = cfg.n_cores * cfg.nblk
    counts = np.bincount(key, minlength=nbuck)
    starts = np.concatenate([[0], np.cumsum(counts)])

    tg, mcs = [], []
    for g in range(cfg.nblk):
        m = max(int(counts[c * cfg.nblk + g]) for c in range(cfg.n_cores))
        m = max(1, m)
        tg.append((m + 127) // 128)
        mcs.append(m)  # gather only this many rows; slots beyond are stale

    # vectorized global row ids
    c_s = src // cfg.npc
    r_s = src % cfg.npc
    w_s = r_s // 128
    ch_s = np.searchsorted(np.array(cfg.cstart), w_s, side="right") - 1
    cw = np.array(cfg.cwin)[ch_s]
    cb = np.array(cfg.cbase)[ch_s]
    cs0 = np.array(cfg.cstart)[ch_s]
    grow_all = cb + c_s * cw * 128 + (w_s - cs0) * 128 + (r_s - w_s * 128)
    dstl_all = (dst % cfg.npc) % 128

    per_core = []
    for c in range(cfg.n_cores):
        gidx_cols, s_cols, d_cols = [], [], []
        for g in range(cfg.nblk):
            b = c * cfg.nblk + g
            s0, s1 = starts[b], starts[b + 1]
            cnt = s1 - s0
            tot = tg[g] * 128
            gp = np.full(tot, cfg.dummy, dtype=np.int64)
            dl = np.zeros(tot, dtype=np.int64)
            gp[:cnt] = grow_all[s0:s1]
            dl[:cnt] = dstl_all[s0:s1]
            wrap = gp.astype(np.int16).reshape(-1, 16).T  # [16, T*8]
            gidx_cols.append(np.tile(wrap, (8, 1)))  # [128, T*8]
            # one-hot S per tile: [128 slots, 128 dstl] (lhsT of segment-sum)
            # slots >= mcs[g] are never gathered: zero column (and the kernel
            # zeroes their ee), so they contribute nothing
            S = np.zeros((tg[g], 128, 128), dtype=ml_dtypes.bfloat16)
            sl = np.arange(mcs[g])
            S[sl // 128, sl % 128, dl[: mcs[g]]] = 1.0
            s_cols.append(S.transpose(1, 0, 2).reshape(128, tg[g] * 128))
            # dl replicated across partitions (device builds S^T via is_equal)
            d_cols.append(
                np.tile(dl.reshape(1, -1), (128, 1)).astype(ml_dtypes.bfloat16)
            )
        per_core.append(
            dict(
                gidx=np.ascontiguousarray(np.concatenate(gidx_cols, axis=1)),
                sall=np.ascontiguousarray(np.concatenate(s_cols, axis=1)),
                dstT=np.ascontiguousarray(np.concatenate(d_cols, axis=1)),
            )
        )
    return tg, mcs, per_core


def split_hilo(a):
    """f64/f32 array -> (bf16 hi, bf16 lo) with hi+lo ~ f32-accurate."""
    hi = a.astype(ml_dtypes.bfloat16)
    lo = (a - hi.astype(np.float64)).astype(ml_dtypes.bfloat16)
    return hi, lo


def make_weight_tensors(W, a_s, a_d):
    """W [fin, fo] f32; a_s/a_d [H, C] -> (wh [fin, fo] bf16,
    wA [fin, 16] bf16 = [hi(W@A) | lo(W@A)])."""
    fin, fo = W.shape
    H, C = a_s.shape
    A = np.zeros((fo, 2 * H), dtype=np.float64)
    for h in range(H):
        A[h * C : (h + 1) * C, h] = a_s[h]
        A[h * C : (h + 1) * C, H + h] = a_d[h]
    WA = W.astype(np.float64) @ A  # [fin, 8]
    hi, lo = split_hilo(WA)
    wh = W.astype(ml_dtypes.bfloat16)
    wA = np.concatenate([hi, lo], axis=1)
    return np.ascontiguousarray(wh), np.ascontiguousarray(wA)


# ---------------------------------------------------------------- bass kernel


def build_kernel(cfg, tg, mcs, dbg=False):
    nblk = cfg.nblk
    ttot = sum(tg)
    L = cfg.n_layers
    nc = bacc.Bacc(
        "TRN2",
        target_bir_lowering=False,
        debug=False,
        num_devices=cfg.n_cores,
        num_swdge_queues=4,
    )
    dbg_d = {}
    if dbg:
        dbg_d["dbg_hrow0"] = nc.dram_tensor(
            "dbg_hrow0", [128, nblk, cfg.rp[0]], BF16, kind="ExternalOutput"
        )
        dbg_d["dbg_et"] = nc.dram_tensor(
            "dbg_et", [128, tg[0], 4], F32, kind="ExternalOutput"
        )
        dbg_d["dbg_xn"] = nc.dram_tensor(
            "dbg_xn", [128, cfg.f_out[0]], F32, kind="ExternalOutput"
        )

    # ---- I/O declarations
    xT0h_d = nc.dram_tensor("xT0h", [cfg.in_feat, cfg.npc_pad], BF16, kind="ExternalInput")
    xT0l_d = nc.dram_tensor("xT0l", [cfg.in_feat, cfg.npc_pad], BF16, kind="ExternalInput")
    wh_d, wA_d, bias_d = [], [], []
    for l in range(L):
        wh_d.append(
            nc.dram_tensor(f"wh{l}", [cfg.f_in[l], cfg.f_out[l]], BF16, kind="ExternalInput")
        )
        wA_d.append(
            nc.dram_tensor(f"wA{l}", [cfg.f_in[l], 16], BF16, kind="ExternalInput")
        )
        bias_d.append(
            nc.dram_tensor(f"bias{l}", [128, cfg.f_out[l]], F32, kind="ExternalInput")
        )
    gidx_d = nc.dram_tensor("gidx", [128, 8 * ttot], I16, kind="ExternalInput")
    eemask_d = nc.dram_tensor("eemask", [128, nblk], BF16, kind="ExternalInput")
    sall_d = nc.dram_tensor("sall", [128, 128 * ttot], BF16, kind="ExternalInput")
    dstT_d = nc.dram_tensor("dstT", [128, 128 * ttot], BF16, kind="ExternalInput")
    fcwn_d = nc.dram_tensor("fcwn", [cfg.npc_pad, 64], F32, kind="ExternalInput")
    fcb_d = nc.dram_tensor("fcb", [1, 1], F32, kind="ExternalInput")
    y_d = nc.dram_tensor("y", [1, cfg.gpc], F32, kind="ExternalOutput")

    h_in, h_glob = [], []
    for l in range(L):
        h_in.append(nc.dram_tensor(f"h_in{l}", [cfg.npc_pad, cfg.rp[l]], BF16))
        h_glob.append(
            nc.dram_tensor(f"h_glob{l}", [cfg.nrows, cfg.rp[l]], BF16, addr_space="Shared")
        )
    p_dram = nc.dram_tensor("p_scratch", [cfg.npc_pad, 1], F32)

    identf_c = nc.inline_tensor(np.eye(128, dtype=np.float32), name="identf_c")
    identb_c = nc.inline_tensor(
        np.eye(128, dtype=np.float32).astype(ml_dtypes.bfloat16), name="identb_c"
    )
    iotac_c = nc.inline_tensor(
        np.arange(128, dtype=np.float32)[:, None].astype(ml_dtypes.bfloat16),
        name="iotac_c",
    )

    rg = [list(range(cfg.n_cores))]
    AG = "AllGather"

    with tile.TileContext(nc) as tc:
        with (
            tc.tile_pool(name="persist", bufs=1) as pp,
            tc.tile_pool(name="work", bufs=3) as wp,
            tc.tile_pool(name="gather", bufs=3) as gp,
            tc.tile_pool(name="pe_pool", bufs=2, space="PSUM") as pep,
            tc.tile_pool(name="pd_pool", bufs=3, space="PSUM") as pdp,
            tc.tile_pool(name="ph_pool", bufs=1, space="PSUM") as php,
            tc.tile_pool(name="pt_pool", bufs=2, space="PSUM") as ptp,
        ):
            # ---- persistent loads
            identf_sb = pp.tile([128, 128], F32, tag="identf")
            nc.sync.dma_start(identf_sb[:], identf_c[:])
            identb_sb = pp.tile([128, 128], BF16, tag="identb")
            nc.sync.dma_start(identb_sb[:], identb_c[:])
            ones_sb = pp.tile([128, 1], F32, tag="ones")
            nc.vector.memset(ones_sb[:], 1.0)
            zeros_sb = pp.tile([128, 1], F32, tag="zeros")
            nc.vector.memset(zeros_sb[:], 0.0)

            xT0h_sb = pp.tile([cfg.in_feat, cfg.npc_pad], BF16, tag="xT0h")
            nc.sync.dma_start(xT0h_sb[:], xT0h_d[:])
            xT0l_sb = pp.tile([cfg.in_feat, cfg.npc_pad], BF16, tag="xT0l")
            nc.sync.dma_start(xT0l_sb[:], xT0l_d[:])

            wh_sb, wA_sb, bias_sb = [], [], []
            for l in range(L):
                fin, fo = cfg.f_in[l], cfg.f_out[l]
                p = min(fin, 128)
                kt = (fin + 127) // 128
                w = pp.tile([p, kt, fo], BF16, tag=f"wh{l}")
                nc.sync.dma_start(w[:], wh_d[l].rearrange("(kt p) f -> p kt f", p=p))
                wh_sb.append(w)
                wa = pp.tile([p, kt, 16], BF16, tag=f"wA{l}")
                nc.sync.dma_start(wa[:], wA_d[l].rearrange("(kt p) f -> p kt f", p=p))
                wA_sb.append(wa)
                b = pp.tile([128, fo], F32, tag=f"bias{l}")
                nc.sync.dma_start(b[:], bias_d[l][:])
                bias_sb.append(b)

            gidx_sb = pp.tile([128, 8 * ttot], I16, tag="gidx")
            nc.sync.dma_start(gidx_sb[:], gidx_d[:])
            sall_sb = pp.tile([128, 128 * ttot], BF16, tag="sall")
            nc.sync.dma_start(sall_sb[:], sall_d[:])
            fcw_sb = pp.tile([128, nblk, 64], F32, tag="fcw")
            nc.sync.dma_start(fcw_sb[:], fcwn_d.rearrange("(b p) f -> p b f", p=128))
            fcb_sb = pp.tile([1, 1], F32, tag="fcb")
            nc.sync.dma_start(fcb_sb[:], fcb_d[:])
            iotac_sb = pp.tile([128, 1], BF16, tag="iotac")
            nc.sync.dma_start(iotac_sb[:], iotac_c[:])
            eemask_sb = pp.tile([128, nblk], BF16, tag="eemask")
            nc.sync.dma_start(eemask_sb[:], eemask_d[:])
            p_sb = pp.tile([128, nblk], F32, tag="p_sb")

            # own-node row buffers + d-score (hi/lo) buffers, per layer
            hrow, dwin, esb = [], [], []
            for l in range(L):
                hrow.append(
                    pp.tile(
                        [128, nblk, cfg.rp[l]], BF16,
                        tag=f"hrow{l}", name=f"hrow{l}",
                    )
                )
                dwin.append(
                    pp.tile([128, nblk, 8], BF16, tag=f"dwin{l}", name=f"dwin{l}")
                )
                esb.append(
                    pp.tile([128, nblk, 4], BF16, tag=f"esb{l}", name=f"esb{l}")
                )

            # ---- pre-zero the hsrc pool slots: slots past mcs[g] are never
            # gathered; stale contents must be finite (never NaN bits)
            for zi in range(3):
                hz = gp.tile(
                    [128, max(tg), cfg.rp[0]], BF16, tag="hsrc", name=f"hz{zi}"
                )
                nc.vector.memset(hz[:], 0.0)

            # ---- dummy rows (once, all layers); drow is f32, bitcast for DMA
            for l in range(L):
                fo = cfg.f_out[l]
                drow = wp.tile([1, cfg.rp[l] // 2], F32, tag="drow")
                nc.vector.memset(drow[:], 0.0)
                nc.vector.memset(drow[0:1, fo // 2 : fo // 2 + 4], DUMMY_S)
                nc.sync.dma_start(
                    h_glob[l][cfg.dummy : cfg.dummy + 1, :],
                    drow[0:1, :].bitcast(BF16),
                )

            def finish_node_row(ln, g, ph_ap, pA_ap):
                """psum h [128, fo] + psum scores [128, 8] -> hrow/dwin + DRAM."""
                fo = cfg.f_out[ln]
                nc.scalar.copy(hrow[ln][:, g, 0:fo], ph_ap)
                nc.scalar.copy(
                    hrow[ln][:, g, fo : fo + 8].bitcast(F32), pA_ap[:, 0:4]
                )
                nc.vector.tensor_copy(dwin[ln][:, g, 0:4], pA_ap[:, 4:8])
                nc.vector.tensor_tensor(
                    out=dwin[ln][:, g, 4:8],
                    in0=pA_ap[:, 4:8],
                    in1=dwin[ln][:, g, 0:4],
                    op=mybir.AluOpType.subtract,
                )
                nc.sync.dma_start(
                    h_in[ln].rearrange("(b p) f -> p b f", p=128)[:, g, :],
                    hrow[ln][:, g, :],
                )

            def ag_chunk(ln, ch):
                cs, ce = cfg.cstart[ch], cfg.cstart[ch + 1]
                crows = (ce - cs) * 128
                nc.gpsimd.collective_compute(
                    AG,
                    mybir.AluOpType.bypass,
                    replica_groups=rg,
                    ins=[h_in[ln][cs * 128 : ce * 128, :]],
                    outs=[
                        h_glob[ln][
                            cfg.cbase[ch] : cfg.cbase[ch] + cfg.n_cores * crows, :
                        ]
                    ],
                )

            # ---- initial node phase: table 0 from x
            for g in range(nblk):
                sl = slice(g * 128, (g + 1) * 128)
                ph = pep.tile([128, cfg.f_out[0]], F32, tag="pe")
                pA = pdp.tile([128, 88], F32, tag="pd")
                nc.tensor.matmul(
                    ph[:], lhsT=xT0h_sb[:, sl], rhs=wh_sb[0][:, 0, :],
                    start=True, stop=True,
                )
                nc.tensor.matmul(
                    pA[:, 0:8], lhsT=xT0h_sb[:, sl], rhs=wA_sb[0][:, 0, 0:8],
                    start=True, stop=False,
                )
                nc.tensor.matmul(
                    pA[:, 0:8], lhsT=xT0h_sb[:, sl], rhs=wA_sb[0][:, 0, 8:16],
                    start=False, stop=False,
                )
                nc.tensor.matmul(
                    pA[:, 0:8], lhsT=xT0l_sb[:, sl], rhs=wA_sb[0][:, 0, 0:8],
                    start=False, stop=True,
                )
                finish_node_row(0, g, ph[:], pA[:])
                for ch in range(len(cfg.cwin)):
                    if g == cfg.cstart[ch + 1] - 1:
                        ag_chunk(0, ch)

            if dbg:
                nc.sync.dma_start(dbg_d["dbg_hrow0"][:], hrow[0][:])

            # ---- layers
            for l in range(L):
                fo = cfg.f_out[l]
                C = fo // HEADS
                rp = cfg.rp[l]
                ln = l + 1
                last = l == L - 1
                if not last:
                    fon = cfg.f_out[ln]
                    ktn = fo // 128  # k-tiles of the next node matmul

                # self-loop scores for all windows of this layer
                dof = wp.tile([128, nblk, 4], F32, tag="dof")
                nc.vector.tensor_tensor(
                    out=dof[:], in0=dwin[l][:, :, 0:4], in1=dwin[l][:, :, 4:8],
                    op=mybir.AluOpType.add,
                )
                esf = wp.tile([128, nblk, 4], F32, tag="esf")
                nc.vector.tensor_tensor(
                    out=esf[:],
                    in0=hrow[l][:, :, fo : fo + 8].bitcast(F32),
                    in1=dof[:],
                    op=mybir.AluOpType.add,
                )
                nc.vector.scalar_tensor_tensor(
                    out=esf[:], in0=esf[:], scalar=0.2, in1=esf[:],
                    op0=mybir.AluOpType.mult, op1=mybir.AluOpType.max,
                )
                nc.scalar.activation(
                    out=esf[:], in_=esf[:], func=mybir.ActivationFunctionType.Exp
                )
                nc.vector.tensor_copy(esb[l][:], esf[:])

                def back(g, pe, pdden):
                    # ---- node phase of window g (deferred one window for
                    # software pipelining: DVE keeps window g+1's edge work
                    # ahead of the PSUM-completion wait here)
                    rec = wp.tile([128, 4], F32, tag="rec", name="rec")
                    den_ap = pdden[:, 80:84] if l == 0 else pe[:, fo : fo + 4]
                    nc.vector.tensor_scalar(
                        out=rec[:], in0=den_ap, scalar1=1e-30,
                        scalar2=None, op0=mybir.AluOpType.add,
                    )
                    nc.vector.reciprocal(rec[:], rec[:])
                    xp = wp.tile([128, fo], F32, tag="xp", bufs=2, name="xp")
                    for h in range(HEADS):
                        nc.vector.scalar_tensor_tensor(
                            out=xp[:, h * C : (h + 1) * C],
                            in0=pe[:, h * C : (h + 1) * C],
                            scalar=rec[:, h : h + 1],
                            in1=bias_sb[l][:, h * C : (h + 1) * C],
                            op0=mybir.AluOpType.mult,
                            op1=mybir.AluOpType.add,
                        )
                    xm = wp.tile([128, fo], F32, tag="xm", bufs=2, name="xm")
                    nc.vector.tensor_tensor(
                        out=xm[:], in0=xp[:],
                        in1=zeros_sb[:, 0:1].to_broadcast([128, fo]),
                        op=mybir.AluOpType.min,
                    )
                    nc.scalar.activation(
                        out=xm[:], in_=xm[:], func=mybir.ActivationFunctionType.Exp
                    )
                    xn = wp.tile([128, fo], F32, tag="xn", bufs=2, name="xn")
                    nc.vector.scalar_tensor_tensor(
                        out=xn[:], in0=xm[:], scalar=-1.0, in1=xp[:],
                        op0=mybir.AluOpType.add, op1=mybir.AluOpType.max,
                    )
                    if dbg and l == 0 and g == 0:
                        nc.sync.dma_start(dbg_d["dbg_xn"][:], xn[:])

                    if not last:
                        # hi/lo split of xn, transpose, next-layer node matmul
                        xnh = wp.tile([128, fo], BF16, tag="xnh", bufs=2, name="xnh")
                        nc.scalar.copy(xnh[:], xn[:])
                        xnhf = wp.tile([128, fo], F32, tag="xnhf", bufs=2, name="xnhf")
                        nc.scalar.copy(xnhf[:], xnh[:])
                        xnl = wp.tile([128, fo], BF16, tag="xnl", bufs=2, name="xnl")
                        nc.vector.tensor_tensor(
                            out=xnl[:], in0=xn[:], in1=xnhf[:],
                            op=mybir.AluOpType.subtract,
                        )
                        xnTh = wp.tile(
                            [128, ktn, 128], BF16, tag="xnTh", bufs=2, name="xnTh"
                        )
                        xnTl = wp.tile(
                            [128, ktn, 128], BF16, tag="xnTl", bufs=2, name="xnTl"
                        )
                        for fb in range(ktn):
                            fsl = slice(fb * 128, (fb + 1) * 128)
                            pth = ptp.tile([128, 128], BF16, tag="pt", name="pth")
                            nc.tensor.transpose(pth[:], xnh[:, fsl], identb_sb[:])
                            nc.scalar.copy(xnTh[:, fb, :], pth[:])
                            ptl = ptp.tile([128, 128], BF16, tag="pt", name="ptl")
                            nc.tensor.transpose(ptl[:], xnl[:, fsl], identb_sb[:])
                            nc.scalar.copy(xnTl[:, fb, :], ptl[:])
                        ph = php.tile([128, 264], F32, tag="ph", name="ph")
                        pA = pdp.tile([128, 88], F32, tag="pd", name="pA")
                        for k in range(ktn):
                            nc.tensor.matmul(
                                ph[:, 0:fon], lhsT=xnTh[:, k, :],
                                rhs=wh_sb[ln][:, k, :],
                                start=(k == 0), stop=(k == ktn - 1),
                            )
                            nc.tensor.matmul(
                                pA[:, 0:8], lhsT=xnTh[:, k, :],
                                rhs=wA_sb[ln][:, k, 0:8],
                                start=(k == 0), stop=False,
                            )
                            nc.tensor.matmul(
                                pA[:, 0:8], lhsT=xnTh[:, k, :],
                                rhs=wA_sb[ln][:, k, 8:16],
                                start=False, stop=False,
                            )
                            nc.tensor.matmul(
                                pA[:, 0:8], lhsT=xnTl[:, k, :],
                                rhs=wA_sb[ln][:, k, 0:8],
                                start=False, stop=(k == ktn - 1),
                            )
                        finish_node_row(ln, g, ph[:, 0:fon], pA[:])
                        for ch in range(len(cfg.cwin)):
                            if g == cfg.cstart[ch + 1] - 1:
                                ag_chunk(ln, ch)
                    else:
                        junk = wp.tile([128, 64], F32, tag="junk", name="junk")
                        nc.vector.scalar_tensor_tensor(
                            out=junk[:],
                            in0=xn[:, 0:64],
                            scalar=1.0,
                            in1=fcw_sb[:, g, :],
                            op0=mybir.AluOpType.mult,
                            op1=mybir.AluOpType.mult,
                            accum_out=p_sb[:, g : g + 1],
                        )

                pending = None
                toff = 0
                for g in range(nblk):
                    T = tg[g]
                    hsrc = gp.tile([128, T, rp], BF16, tag="hsrc")
                    nc.gpsimd.dma_gather(
                        out_ap=hsrc[:],
                        in_ap=h_glob[l][:],
                        idxs_ap=gidx_sb[:, 8 * toff : 8 * (toff + T)],
                        num_idxs=T * 128,
                        num_idxs_reg=int(mcs[g]),
                        elem_size=rp,
                        single_packet=False,
                        queue_num=g % 4,
                    )
                    dstT_sb = gp.tile([128, T * 128], BF16, tag="dstT", bufs=3)
                    nc.sync.dma_start(
                        dstT_sb[:], dstT_d[:, 128 * toff : 128 * (toff + T)]
                    )
                    # self-loop message (+ ee columns for the merged den)
                    msgs = wp.tile([128, fo + 4], BF16, tag="msgs", bufs=4)
                    es_g = esb[l][:, g, :]
                    es_b = bass.AP(es_g.tensor, es_g.offset, list(es_g.ap) + [[0, C]])
                    nc.vector.tensor_tensor(
                        out=msgs[:, 0:fo].rearrange("p (h c) -> p h c", h=HEADS),
                        in0=hrow[l][:, g, 0:fo].rearrange("p (h c) -> p h c", h=HEADS),
                        in1=es_b,
                        op=mybir.AluOpType.mult,
                    )
                    nc.scalar.copy(msgs[:, fo : fo + 4], esb[l][:, g, :])
                    pdden = pdp.tile([128, 88], F32, tag="pd")
                    for t in range(T):
                        St = wp.tile([128, 128], BF16, tag="St", bufs=9)
                        nc.vector.tensor_tensor(
                            out=St[:],
                            in0=dstT_sb[:, 128 * t : 128 * (t + 1)],
                            in1=iotac_sb[:, 0:1].to_broadcast([128, 128]),
                            op=mybir.AluOpType.is_equal,
                        )
                        nc.tensor.matmul(
                            pdden[:, t * 8 : t * 8 + 8], lhsT=St[:],
                            rhs=dwin[l][:, g, :], start=True, stop=True,
                        )
                    pdv = pdden[:, 0 : T * 8].rearrange("p (t e) -> p t e", e=8)
                    et = wp.tile([128, T, 4], F32, tag="et", bufs=4)
                    nc.vector.tensor_tensor(
                        out=et[:],
                        in0=hsrc[:, :, fo : fo + 8].bitcast(F32),
                        in1=pdv[:, :, 0:4],
                        op=mybir.AluOpType.add,
                    )
                    nc.vector.tensor_tensor(
                        out=et[:], in0=et[:], in1=pdv[:, :, 4:8],
                        op=mybir.AluOpType.add,
                    )
                    nc.vector.scalar_tensor_tensor(
                        out=et[:], in0=et[:], scalar=0.2, in1=et[:],
                        op0=mybir.AluOpType.mult, op1=mybir.AluOpType.max,
                    )
                    nc.scalar.activation(
                        out=et[:], in_=et[:], func=mybir.ActivationFunctionType.Exp
                    )
                    if dbg and l == 0 and g == 0:
                        nc.sync.dma_start(dbg_d["dbg_et"][:], et[:])
                    msg = wp.tile([128, T, fo + 4], BF16, tag="msg", bufs=2)
                    eebf = msg[:, :, fo : fo + 4]
                    nc.scalar.copy(eebf, et[:])
                    if mcs[g] < T * 128:
                        # zero ee of never-gathered stale slots (last tile)
                        mk = eemask_sb[:, g : g + 1]
                        nc.vector.tensor_tensor(
                            out=msg[:, T - 1, fo : fo + 4],
                            in0=msg[:, T - 1, fo : fo + 4],
                            in1=bass.AP(mk.tensor, mk.offset, list(mk.ap[:1]) + [[0, 4]]),
                            op=mybir.AluOpType.mult,
                        )
                    for t in range(T):
                        eh = msg[:, t, fo : fo + 4]
                        eb = bass.AP(eh.tensor, eh.offset, list(eh.ap) + [[0, C]])
                        nc.vector.tensor_tensor(
                            out=msg[:, t, 0:fo].rearrange("p (h c) -> p h c", h=HEADS),
                            in0=hsrc[:, t, 0:fo].rearrange("p (h c) -> p h c", h=HEADS),
                            in1=eb,
                            op=mybir.AluOpType.mult,
                        )
                    pe = pep.tile([128, cfg.f_out[0]], F32, tag="pe")
                    if l == 0:
                        # fo+4 > one PSUM bank: keep the denominator separate
                        for t in range(T):
                            St = sall_sb[:, (toff + t) * 128 : (toff + t + 1) * 128]
                            nc.tensor.matmul(
                                pe[:, 0:fo], lhsT=St, rhs=msg[:, t, 0:fo],
                                start=(t == 0), stop=False,
                            )
                            nc.tensor.matmul(
                                pdden[:, 80:84], lhsT=St, rhs=msg[:, t, fo : fo + 4],
                                start=(t == 0), stop=False,
                            )
                        nc.tensor.matmul(
                            pe[:, 0:fo], lhsT=identb_sb[:], rhs=msgs[:, 0:fo],
                            start=False, stop=True,
                        )
                        nc.tensor.matmul(
                            pdden[:, 80:84], lhsT=identb_sb[:], rhs=msgs[:, fo : fo + 4],
                            start=False, stop=True,
                        )
                    else:
                        for t in range(T):
                            St = sall_sb[:, (toff + t) * 128 : (toff + t + 1) * 128]
                            nc.tensor.matmul(
                                pe[:, 0 : fo + 4], lhsT=St, rhs=msg[:, t, :],
                                start=(t == 0), stop=False,
                            )
                        nc.tensor.matmul(
                            pe[:, 0 : fo + 4], lhsT=identb_sb[:], rhs=msgs[:],
                            start=False, stop=True,
                        )
                    if pending is not None:
                        back(*pending)
                    pending = (g, pe, pdden)
                    toff += T
                back(*pending)

            # ---- readout: per-graph sums of p over npg-node segments
            nc.sync.dma_start(
                p_dram.rearrange("(b p) one -> p (b one)", p=128), p_sb[:]
            )
            pw = min(128, cfg.npg)
            pa = pp.tile([pw, cfg.gpc], F32, tag="pa")
            pd_ap = p_dram[:]
            nc.sync.dma_start(
                pa[:], bass.AP(pd_ap.tensor, 0, [[1, pw], [cfg.npg, cfg.gpc]])
            )
            rem = cfg.npg - 128
            if rem > 0:
                pb = pp.tile([128, cfg.gpc], F32, tag="pb")
                nc.sync.dma_start(
                    pb[0:rem, :],
                    bass.AP(pd_ap.tensor, 128, [[1, rem], [cfg.npg, cfg.gpc]]),
                )
            yp = ptp.tile([1, cfg.gpc], F32, tag="pt")
            nc.tensor.matmul(
                yp[0:1, :], lhsT=ones_sb[0:pw, 0:1], rhs=pa[:],
                start=True, stop=(rem <= 0),
            )
            if rem > 0:
                nc.tensor.matmul(
                    yp[0:1, :], lhsT=ones_sb[0:rem, 0:1], rhs=pb[0:rem, :],
                    start=False, stop=True,
                )
            y_sb = pp.tile([1, cfg.gpc], F32, tag="y_sb")
            nc.vector.tensor_scalar(
                out=y_sb[:], in0=yp[0:1, :], scalar1=fcb_sb[0:1, 0:1],
                scalar2=None, op0=mybir.AluOpType.add,
            )
            nc.sync.dma_start(y_d[:], y_sb[:])

    nc.compile()
    return nc


# ------------------------------------------------------------------- driver

last_results = None
_cache = {}


def _prepare(cfg, inputs):
    tg, mcs, per_core = preprocess_edges(cfg, np.asarray(inputs["edge_index"]))
    x = np.asarray(inputs["x"], dtype=np.float32)
    fcw = np.asarray(inputs["fcw"], dtype=np.float32)
    fcb = np.asarray(inputs["fcb"], dtype=np.float32).reshape(1, 1)
    whs, wAs, biases = [], [], []
    for l in range(cfg.n_layers):
        wh, wA = make_weight_tensors(
            np.asarray(inputs[f"W{l + 1}"], np.float32),
            np.asarray(inputs[f"as{l + 1}"], np.float32),
            np.asarray(inputs[f"ad{l + 1}"], np.float32),
        )
        whs.append(wh)
        wAs.append(wA)
        biases.append(
            np.tile(np.asarray(inputs[f"b{l + 1}"], np.float32)[None, :], (128, 1))
        )
    fcw_node_full = fcw.reshape(cfg.npg, 64)[np.arange(cfg.n_nodes) % cfg.npg]

    in_maps = []
    for c in range(cfg.n_cores):
        xs = x[c * cfg.npc : (c + 1) * cfg.npc]
        xT0 = np.zeros((cfg.in_feat, cfg.npc_pad), np.float64)
        xT0[:, : cfg.npc] = xs.T
        xh, xl = split_hilo(xT0)
        fcwn = np.zeros((cfg.npc_pad, 64), np.float32)
        fcwn[: cfg.npc] = fcw_node_full[c * cfg.npc : (c + 1) * cfg.npc]
        eemask = np.zeros((128, cfg.nblk), dtype=ml_dtypes.bfloat16)
        for g in range(cfg.nblk):
            base = ((mcs[g] + 127) // 128 - 1) * 128
            eemask[:, g] = [1.0 if base + p < mcs[g] else 0.0 for p in range(128)]
        m = dict(
            eemask=np.ascontiguousarray(eemask),
            xT0h=np.ascontiguousarray(xh),
            xT0l=np.ascontiguousarray(xl),
            gidx=per_core[c]["gidx"],
            sall=per_core[c]["sall"],
            dstT=per_core[c]["dstT"],
            fcwn=np.ascontiguousarray(fcwn),
            fcb=fcb,
        )
        for l in range(cfg.n_layers):
            m[f"wh{l}"] = whs[l]
            m[f"wA{l}"] = wAs[l]
            m[f"bias{l}"] = biases[l]
        in_maps.append(m)
    return tg, mcs, in_maps


def _ensure_ntff_hook():
    """Shim antenv.axon_hooks (absent in this image) so BASS_TRACE works."""
    try:
        from antenv.axon_hooks import get_axon_ntff_profile_hook  # noqa: F401

        return
    except ImportError:
        pass
    try:
        import types

        import antenv

        mod = types.ModuleType("antenv.axon_hooks")
        holder = [None]
        mod.set_axon_ntff_profile_hook = lambda h: holder.__setitem__(0, h)
        mod.get_axon_ntff_profile_hook = lambda: holder[0]
        sys.modules["antenv.axon_hooks"] = mod
        antenv.axon_hooks = mod
        from trn_agent_boot.trn_boot import _ntff_profile_via_ctypes

        h = _ntff_profile_via_ctypes("/opt/axon/libaxon_pjrt.so")
        if h is not None:
            holder[0] = h
    except Exception:
        pass


def run(cfg, inputs, trace=False, dbg=False):
    global last_results
    if trace or os.environ.get("BASS_TRACE"):
        _ensure_ntff_hook()
    tg, mcs, in_maps = _prepare(cfg, inputs)
    key = (cfg.n_nodes, tuple(tg), tuple(mcs), dbg)
    if key not in _cache:
        _cache[key] = build_kernel(cfg, tg, mcs, dbg=dbg)
    nc = _cache[key]
    res = run_bass_kernel_spmd(
        nc, in_maps, core_ids=list(range(cfg.n_cores)), trace=trace
    )
    last_results = res
    y = np.concatenate([r["y"].reshape(-1) for r in res.results])
    return y.reshape(-1, 1).astype(np.float32)


def kernel(**inputs) -> np.ndarray:
    cfg = default_cfg()
    return run(cfg, inputs)
